# revision 31
# baseline (speedup 1.0000x reference)
"""ArcFace inner-product kernel for one TRN2 chip (8 NeuronCores).

Problem: feat [4096, 512] f32, label [4096] i64, weights [20000, 512] f32.
  nf = l2norm(feat, axis=1); nw = l2norm(weights, axis=1)
  cos = nf @ nw.T                               [4096, 20000]
  ml  = 30 * cos(arccos(cos) + margin-at-label) [4096, 20000]
Returns (cos, ml).

Sharding: tensor-parallel over the class dim C. Each core receives the
full feat plus a 2500-row slice of weights and produces the matching
2500-column slices of both outputs. No collectives: the per-row label
fixup touches only 4096 of the 82M output elements and is applied on the
host after the column-slice gather.

Device kernel per core:
  1. weight tiles [128, 512]: square+row-reduce (ACT Square + accum_out),
     rsqrt (DVE reciprocal + ACT sqrt), row-scale (ACT), PE-transpose
     into K-major SBUF tiles ([K-chunk, class-cols], float32r).
  2. feat tiles: same minus the row-scale - the feat norm is applied
     later as the per-partition ACT scale of the PSUM evict.
  3. For each output tile [128, n<=512]: 4 accumulating matmuls over K
     (float32r: the PE runs 1 cycle/row vs 4 for plain fp32), evict
     cos = rf*psum (ACT) and ml = 30*cos (DVE) into [128, 2500] strips,
     DMA strips out (1.25MB contiguous writes).
"""

import math

import numpy as np

from concourse import bacc, mybir, tile
from concourse.masks import make_identity
from concourse.bass_utils import run_bass_kernel_spmd

B, D, C = 4096, 512, 20000
NCORES = 8
CLOC = C // NCORES  # 2500
KCH = D // 128      # 4 k-chunks
MT = B // 128       # 32 B-tiles
NT = (CLOC + 511) // 512  # 5 n-chunks (last = 452)
WT = (CLOC + 127) // 128  # 20 w-tiles (last = 68 rows)

SCALE = 30.0
MARGIN = 0.5
THRESH = -math.cos(MARGIN)
EXT_VAL = -MARGIN * math.sin(MARGIN)
COS_M = math.cos(MARGIN)
SIN_M = math.sin(MARGIN)

F32 = mybir.dt.float32
F32R = mybir.dt.float32r

_NC_CACHE = {}


def _build_nc(repeats=1):
    nc = bacc.Bacc(
        "TRN2",
        target_bir_lowering=False,
        debug=False,
        num_devices=NCORES,
    )
    feat = nc.dram_tensor("feat", [B, D], F32, kind="ExternalInput").ap()
    w = nc.dram_tensor("w", [CLOC, D], F32, kind="ExternalInput").ap()
    cos_o = nc.dram_tensor("cos_o", [B, CLOC], F32, kind="ExternalOutput").ap()
    ml_o = nc.dram_tensor("ml_o", [B, CLOC], F32, kind="ExternalOutput").ap()

    with tile.TileContext(nc) as tc:
        with (
            tc.tile_pool(name="const", bufs=1) as const_pool,
            tc.tile_pool(name="persist", bufs=1) as persist,
            tc.tile_pool(name="stage", bufs=6) as stage,
            tc.tile_pool(name="scratch", bufs=4) as scratch,
            tc.tile_pool(name="outs", bufs=3) as outs,
            tc.tile_pool(name="mm_psum", bufs=4, space="PSUM") as mm_psum,
            tc.tile_pool(name="tp_psum", bufs=4, space="PSUM") as tp_psum,
        ):
            ident = const_pool.tile([128, 128], F32, tag="ident")
            make_identity(nc, ident[:])
            NSZ = [min(512, CLOC - n * 512) for n in range(NT)]

            def body(rep):
                sfx = f"_r{rep}" if rep else ""

                # K-major SBUF copies of normalized-transposed operands, one
                # tile per m-strip / n-chunk (k-chunks side by side inside
                # each tile) so Tile's per-tile dependency tracking pipelines
                # prep with the matmuls, and each prep needs only one
                # PSUM->SBUF copy.
                nfT = [
                    persist.tile([128, KCH * 128], F32R, tag=f"nfT{m}",
                                 name=f"nfT{m}{sfx}")
                    for m in range(MT)
                ]
                nwT = [
                    persist.tile([128, KCH * NSZ[n]], F32R, tag=f"nwT{n}",
                                 name=f"nwT{n}{sfx}")
                    for n in range(NT)
                ]

                # Per-m-tile reciprocal feat row norms; applied at PSUM evict
                # as a per-partition ACT scale instead of scaling feat itself.
                rf = [
                    persist.tile([128, 1], F32, tag=f"rf{m}", name=f"rf{m}{sfx}")
                    for m in range(MT)
                ]
                emit(rep, nfT, nwT, rf)

            def rnorm(xt, rows, r_out):
                """r_out[p] = 1/||xt[p,:]|| for the first `rows` partitions.
                (ACT Square+accum_out; vector.tensor_tensor_reduce faults the
                hardware in this environment - do not use it.)"""
                sq = scratch.tile([128, D], F32, tag="sq")
                n2 = scratch.tile([128, 1], F32, tag="n2")
                nc.scalar.activation(
                    sq[:rows],
                    xt[:rows],
                    mybir.ActivationFunctionType.Square,
                    accum_out=n2[:rows],
                )
                rinv = scratch.tile([128, 1], F32, tag="rinv")
                nc.vector.reciprocal(rinv[:rows], n2[:rows])
                nc.scalar.sqrt(r_out[:rows], rinv[:rows])

            def transpose_blocks(xt, rows):
                """Transpose the 4 [rows,128] blocks of xt into one PSUM bank
                laid out k-major; returns the [128, KCH*128] PSUM tile."""
                tp = tp_psum.tile([128, KCH * 128], F32, tag="tp")
                for k in range(KCH):
                    nc.tensor.transpose(
                        tp[:, k * 128 : k * 128 + rows],
                        xt[:rows, k * 128 : (k + 1) * 128],
                        ident[:rows, :rows],
                    )
                return tp

            def emit(rep, nfT, nwT, rf):
                sfx = f"_r{rep}" if rep else ""

                # Weight prep: explicit row-normalize (transpose mode only
                # accepts permutation matrices, so the norm can't ride the
                # transpose).
                for t in range(WT):
                    n = t // 4
                    off = (t % 4) * 128
                    rows = min(128, CLOC - t * 128)
                    xt = stage.tile([128, D], F32, tag="xt", name=f"xtw{t}{sfx}")
                    nc.sync.dma_start(
                        out=xt[:rows], in_=w[t * 128 : t * 128 + rows, :]
                    )
                    rw = scratch.tile([128, 1], F32, tag="rw")
                    rnorm(xt, rows, rw)
                    nc.scalar.mul(xt[:rows], xt[:rows], rw[:rows])
                    tp = transpose_blocks(xt, rows)
                    src = tp[:].rearrange("p (k c) -> p k c", k=KCH)[:, :, :rows]
                    dst = nwT[n][:].rearrange("p (k c) -> p k c", k=KCH)[
                        :, :, off : off + rows
                    ]
                    nc.vector.tensor_copy(dst, src)

                # Feat prep: raw transpose; row norm saved per m-tile.
                def feat_prep(t):
                    xt = stage.tile([128, D], F32, tag="xt", name=f"xtf{t}{sfx}")
                    nc.sync.dma_start(
                        out=xt[:], in_=feat[t * 128 : (t + 1) * 128, :]
                    )
                    rnorm(xt, 128, rf[t])
                    tp = transpose_blocks(xt, 128)
                    nc.vector.tensor_copy(nfT[t][:], tp[:])

                # Interleave feat prep with the main loop at a prefetch
                # distance of PF strips, so f-tile loads queue ahead of the
                # output DMAs that would otherwise block them in the HWDGE
                # FIFO.
                PF = 4
                for t in range(min(PF, MT)):
                    feat_prep(t)

                for m in range(MT):
                    if m + PF < MT:
                        feat_prep(m + PF)
                    cos_strip = outs.tile([128, CLOC], F32, tag="cos_strip")
                    ml_strip = outs.tile([128, CLOC], F32, tag="ml_strip")
                    for n in range(NT):
                        nsz = NSZ[n]
                        ps = mm_psum.tile([128, 512], F32, tag="mm")
                        for k in range(KCH):
                            nc.tensor.matmul(
                                ps[:, :nsz],
                                lhsT=nfT[m][:, k * 128 : (k + 1) * 128],
                                rhs=nwT[n][:, k * nsz : (k + 1) * nsz],
                                start=(k == 0),
                                stop=(k == KCH - 1),
                            )
                        cs = cos_strip[:, n * 512 : n * 512 + nsz]
                        nc.scalar.activation(
                            cs, ps[:, :nsz],
                            mybir.ActivationFunctionType.Copy,
                            scale=rf[m][:],
                        )
                        nc.vector.tensor_scalar_mul(
                            ml_strip[:, n * 512 : n * 512 + nsz], cs, SCALE
                        )
                    nc.sync.dma_start(
                        out=cos_o[m * 128 : (m + 1) * 128, :], in_=cos_strip[:]
                    )
                    nc.sync.dma_start(
                        out=ml_o[m * 128 : (m + 1) * 128, :], in_=ml_strip[:]
                    )

            for rep in range(repeats):
                body(rep)

    nc.compile()
    return nc


def _get_nc():
    if "nc" not in _NC_CACHE:
        _NC_CACHE["nc"] = _build_nc()
    return _NC_CACHE["nc"]


def make_in_maps(feat, weights):
    feat = np.ascontiguousarray(np.asarray(feat, dtype=np.float32))
    weights = np.ascontiguousarray(np.asarray(weights, dtype=np.float32))
    return [
        {"feat": feat, "w": weights[k * CLOC : (k + 1) * CLOC]}
        for k in range(NCORES)
    ]


def assemble(results, label):
    """Gather per-core column slices and apply the per-row label fixup."""
    cos = np.empty((B, C), np.float32)
    ml = np.empty((B, C), np.float32)
    for k in range(NCORES):
        cos[:, k * CLOC : (k + 1) * CLOC] = results[k]["cos_o"]
        ml[:, k * CLOC : (k + 1) * CLOC] = results[k]["ml_o"]
    idx = np.arange(B)
    lab = np.asarray(label).astype(np.int64)
    cil = cos[idx, lab]
    sin_il = np.sqrt(np.maximum(0.0, 1.0 - cil * cil)).astype(np.float32)
    hit = cil > THRESH
    ml[idx, lab] = np.where(
        hit,
        SCALE * (cil * COS_M - sin_il * SIN_M),
        SCALE * (cil + EXT_VAL),
    ).astype(np.float32)
    return cos, ml


def kernel(feat, label, weights):
    nc = _get_nc()
    in_maps = make_in_maps(feat, weights)
    res = run_bass_kernel_spmd(nc, in_maps, core_ids=list(range(NCORES)))
    return assemble(res.results, label)
